# revision 24
# baseline (speedup 1.0000x reference)
"""Trainium2 Bass kernel: causal multi-head self-attention with RoPE.

Problem: x[2,2048,1024], 16 heads, d_k=64, causal, RoPE(theta=1e4),
out = (softmax(rope(Q)rope(K)^T/8) V) WO^T.

Sharding (8 cores): data-parallel over batch (2) x head-parallel over
head groups (4 heads per core).  Each core computes Q/K/V projections
for its 4 heads, flash-style causal attention, and a partial output
projection over its 256 channels; the host sums the 4 partials per
batch element.

Device layouts (per core, all bf16 except PSUM):
  xt  [1024,2048]  x[b]^T (d_model on partitions)
  Qt/Kt tiles [128,2048]: 2 heads each, per head rows = [32 even-dim,
      32 odd-dim] (host permutes W_Q/W_K columns) so RoPE is pure
      row-block ops; scores are permutation-invariant.
  V   [128,16,4,65]: natural [s,d] layout per 128-row s-block, 65th
      column of ones => P@[V|1] yields softmax denominators for free.
  scores computed transposed ([keys,queries]) so P^T feeds P@V with the
      contraction (keys) on partitions.
"""

import os
import sys

for _p in ("/opt/trn_rl_repo",):
    if _p not in sys.path:
        sys.path.insert(0, _p)

import numpy as np
import ml_dtypes

BF16 = ml_dtypes.bfloat16

D = 1024
S = 2048
H = 16
DK = 64
HPC = 4          # heads per core
NCORES = 8
THETA = 10000.0

_COMPILED = {}


def _build_nc():
    import concourse.bass as bass  # noqa: F401
    import concourse.bacc as bacc
    import concourse.mybir as mybir
    import concourse.tile as tile

    bf16 = mybir.dt.bfloat16
    f32 = mybir.dt.float32
    Exp = mybir.ActivationFunctionType.Exp

    nc = bacc.Bacc(
        "TRN2", target_bir_lowering=False, debug=False, num_devices=NCORES
    )
    xt_d = nc.declare_dram_parameter("xt", [D, S], bf16, isOutput=False)
    wq_d = nc.declare_dram_parameter("wq", [D, 256], bf16, isOutput=False)
    wk_d = nc.declare_dram_parameter("wk", [D, 256], bf16, isOutput=False)
    wv_d = nc.declare_dram_parameter("wv", [D, 256], bf16, isOutput=False)
    wo_d = nc.declare_dram_parameter("wo", [256, D], bf16, isOutput=False)
    cos_d = nc.declare_dram_parameter("cosb", [128, S], bf16, isOutput=False)
    sin_d = nc.declare_dram_parameter("sinb", [128, S], bf16, isOutput=False)
    msk_d = nc.declare_dram_parameter("msk", [128, 4, 512], bf16, isOutput=False)
    eye_d = nc.declare_dram_parameter("eye", [128, 128], bf16, isOutput=False)
    ind_d = nc.declare_dram_parameter("ind", [8, 4, 128], bf16, isOutput=False)
    out_d = nc.declare_dram_parameter("out", [S, D], f32, isOutput=True)

    with tile.TileContext(nc) as tc:
        with tc.tile_pool(name="const", bufs=1) as const:
            x_sb = const.tile([128, 8, S], bf16)
            wq_sb = const.tile([128, 8, 256], bf16)
            wk_sb = const.tile([128, 8, 256], bf16)
            wv_sb = const.tile([128, 8, 256], bf16)
            wo_sb = const.tile([128, 2, D], bf16)
            cos_sb = const.tile([128, S], bf16)
            sin_sb = const.tile([128, S], bf16)
            msk_sb = const.tile([128, 4, 512], bf16)
            eye_sb = const.tile([128, 128], bf16)
            ind_sb = const.tile([8, 4, 128], bf16)
            v_sb = const.tile([128, 16, 4, 65], bf16)
            qraw = [const.tile([128, S], bf16, name=f"qraw{i}") for i in range(2)]
            kraw = [const.tile([128, S], bf16, name=f"kraw{i}") for i in range(2)]
            qrot = [const.tile([128, S], bf16, name=f"qrot{i}") for i in range(2)]
            krot = [const.tile([128, S], bf16, name=f"krot{i}") for i in range(2)]
            at = [const.tile([128, S], bf16, name=f"at{i}") for i in range(2)]

            nc.sync.dma_start(wq_sb[:], wq_d[:].rearrange("(c p) m -> p c m", p=128))
            nc.sync.dma_start(wk_sb[:], wk_d[:].rearrange("(c p) m -> p c m", p=128))
            nc.sync.dma_start(wv_sb[:], wv_d[:].rearrange("(c p) m -> p c m", p=128))
            nc.sync.dma_start(wo_sb[:], wo_d[:].rearrange("(c p) m -> p c m", p=128))
            nc.sync.dma_start(cos_sb[:], cos_d[:])
            nc.sync.dma_start(sin_sb[:], sin_d[:])
            nc.sync.dma_start(msk_sb[:], msk_d[:])
            nc.sync.dma_start(eye_sb[:], eye_d[:])
            nc.sync.dma_start(ind_sb[:], ind_d[:])
            for nsl in range(4):
                nc.sync.dma_start(
                    x_sb[:, :, nsl * 512:(nsl + 1) * 512],
                    xt_d[:, nsl * 512:(nsl + 1) * 512].rearrange(
                        "(c p) s -> p c s", p=128
                    ),
                )
            nc.vector.memset(v_sb[:, :, :, 64:65], 1.0)

            # ---- phase 1: Q/K/V projections ----
            with tc.tile_pool(name="pj", bufs=4, space="PSUM") as pjp, \
                 tc.tile_pool(name="pvps", bufs=2, space="PSUM") as pvps:
                for w_sb, raw in ((wq_sb, qraw), (wk_sb, kraw)):
                    for ot in range(2):
                        for nsl in range(4):
                            ps = pjp.tile([128, 512], f32, tag="pj")
                            for c in range(8):
                                nc.tensor.matmul(
                                    ps[:],
                                    w_sb[:, c, ot * 128:(ot + 1) * 128],
                                    x_sb[:, c, nsl * 512:(nsl + 1) * 512],
                                    start=(c == 0), stop=(c == 7),
                                )
                            nc.scalar.copy(raw[ot][:, nsl * 512:(nsl + 1) * 512], ps[:])
                for sb in range(16):
                    ps = pvps.tile([128, 256], f32, tag="pv")
                    for c in range(8):
                        nc.tensor.matmul(
                            ps[:],
                            x_sb[:, c, sb * 128:(sb + 1) * 128],
                            wv_sb[:, c, :],
                            start=(c == 0), stop=(c == 7),
                        )
                    nc.vector.tensor_copy(
                        v_sb[:, sb, :, 0:64],
                        ps[:].rearrange("p (h d) -> p h d", h=4),
                    )

                # ---- RoPE on Q and K ----
                with tc.tile_pool(name="rope", bufs=2) as rp:
                    for raw, rot in ((qraw, qrot), (kraw, krot)):
                        for ot in range(2):
                            sw = rp.tile([128, S], bf16, tag="sw")
                            t1 = rp.tile([128, S], bf16, tag="t1")
                            for blk in range(4):
                                src = blk ^ 1
                                nc.sync.dma_start(
                                    sw[blk * 32:(blk + 1) * 32, :],
                                    raw[ot][src * 32:(src + 1) * 32, :],
                                )
                            nc.vector.tensor_mul(t1[:], raw[ot][:], cos_sb[:])
                            nc.vector.tensor_mul(sw[:], sw[:], sin_sb[:])
                            nc.vector.tensor_add(rot[ot][:], t1[:], sw[:])

            # ---- phase 2: causal attention (scores transposed) ----
            # at[] collects UNNORMALIZED head outputs (bf16); den_sb collects
            # the 16 denominator rows.  Normalization for head-pair `ot` is
            # emitted as soon as its two heads finish, overlapping the other
            # pair's attention; normalized slices land in atn for outproj.
            den_sb = [const.tile([8, 512], bf16, name=f"den{i}")
                      for i in range(2)]
            rc = [const.tile([8, 512], f32, name=f"rc{i}") for i in range(2)]
            rcb = [const.tile([8, 512], bf16, name=f"rcb{i}") for i in range(2)]
            atn = [const.tile([128, 4, 512], bf16, name=f"atn{i}")
                   for i in range(2)]
            with tc.tile_pool(name="ps_s", bufs=2, space="PSUM") as psc, \
                 tc.tile_pool(name="ps_o", bufs=2, space="PSUM") as pso, \
                 tc.tile_pool(name="ps_r", bufs=2, space="PSUM") as psr, \
                 tc.tile_pool(name="pp", bufs=3) as ppool, \
                 tc.tile_pool(name="nrm", bufs=3) as nrm:
                for h in range(HPC):
                    ot, hl = divmod(h, 2)
                    qr, kr = qrot[ot], krot[ot]
                    r0 = hl * 64
                    for j in range(4):
                        nkb = 4 * (j + 1)
                        po = pso.tile([65, 512], f32, tag="po")
                        for g0 in range(0, nkb, 2):
                            G = min(2, nkb - g0)
                            sp = psc.tile([128, 1024], f32, tag="sc")
                            pt = ppool.tile([128, 1024], bf16, tag="pt")
                            for i in range(G):
                                kb = g0 + i
                                dg = kb - 4 * j
                                if dg >= 0:
                                    # causal mask: preload psum with -1e5 in
                                    # the key>query region via identity matmul
                                    nc.tensor.matmul(
                                        sp[:, i * 512:(i + 1) * 512],
                                        eye_sb[:],
                                        msk_sb[:, dg, :],
                                        start=True, stop=False,
                                    )
                                nc.tensor.matmul(
                                    sp[:, i * 512:(i + 1) * 512],
                                    kr[r0:r0 + 64, kb * 128:(kb + 1) * 128],
                                    qr[r0:r0 + 64, j * 512:(j + 1) * 512],
                                    start=(dg < 0), stop=True,
                                )
                            nc.scalar.activation(
                                pt[:, 0:G * 512], sp[:, 0:G * 512], Exp, scale=0.125
                            )
                            for i in range(G):
                                kb = g0 + i
                                nc.tensor.matmul(
                                    po[:],
                                    v_sb[:, kb, h, 0:65],
                                    pt[:, i * 512:(i + 1) * 512],
                                    start=(kb == 0), stop=(kb == nkb - 1),
                                )
                        # stage unnormalized out + denominator, release po fast
                        tm = nrm.tile([65, 512], bf16, tag="tm")
                        nc.vector.tensor_copy(tm[:], po[:])
                        nc.sync.dma_start(
                            at[ot][r0:r0 + 64, j * 512:(j + 1) * 512], tm[0:64, :]
                        )
                        nc.sync.dma_start(
                            den_sb[ot][hl * 4 + j:hl * 4 + j + 1, :], tm[64:65, :]
                        )
                    if hl == 1:
                        # both heads of pair `ot` finished: normalize its
                        # at[] half now (overlaps the next pair's attention)
                        nc.vector.reciprocal(rc[ot][:], den_sb[ot][:])
                        nc.vector.tensor_copy(rcb[ot][:], rc[ot][:])
                        for jsl in range(4):
                            rbp = psr.tile([128, 512], f32, tag="rb")
                            nc.tensor.matmul(
                                rbp[:], ind_sb[:, jsl, :],
                                rcb[ot][:], start=True, stop=True,
                            )
                            nc.vector.tensor_mul(
                                atn[ot][:, jsl, :],
                                at[ot][:, jsl * 512:(jsl + 1) * 512],
                                rbp[:],
                            )

            # ---- phase 3: partial output projection ----
            with tc.tile_pool(name="ps_f", bufs=4, space="PSUM") as psf, \
                 tc.tile_pool(name="ost", bufs=4) as ost:
                for sb in range(16):
                    jsl, sbi = divmod(sb, 4)
                    for osl in range(2):
                        pf = psf.tile([128, 512], f32, tag="pf")
                        for ich in range(2):
                            nc.tensor.matmul(
                                pf[:],
                                atn[ich][:, jsl, sbi * 128:(sbi + 1) * 128],
                                wo_sb[:, ich, osl * 512:(osl + 1) * 512],
                                start=(ich == 0), stop=(ich == 1),
                            )
                        ob = ost.tile([128, 512], f32, tag="ob")
                        nc.vector.tensor_copy(ob[:], pf[:])
                        nc.sync.dma_start(
                            out_d[sb * 128:(sb + 1) * 128, osl * 512:(osl + 1) * 512],
                            ob[:],
                        )
    nc.compile()
    return nc


def _host_prep(x, token_positions, WQ, WK, WV, WO):
    """Build the 8 per-core input maps."""
    pos = np.asarray(token_positions).astype(np.float32)
    k = np.arange(DK // 2, dtype=np.float32)
    inv_freq = 1.0 / (THETA ** (2.0 * k / DK))
    ang = pos[:, None] * inv_freq[None, :]          # [S, 32]
    c32 = np.cos(ang).T.astype(np.float32)          # [32, S]
    s32 = np.sin(ang).T.astype(np.float32)
    cosb = np.tile(c32, (4, 1)).astype(BF16)        # [128, S]
    sinb = np.concatenate([-s32, s32, -s32, s32], axis=0).astype(BF16)
    # causal masks for the 4 diagonal key-blocks of a 512-query slice
    kk = np.arange(128)[:, None, None]
    dd = np.arange(4)[None, :, None]
    qq = np.arange(512)[None, None, :]
    msk = np.where(dd * 128 + kk <= qq, 0.0, -1e5).astype(BF16)  # [128, 4, 512]
    eye = np.eye(128, dtype=np.float32).astype(BF16)
    # indicator matrices for denominator broadcast:
    # ind[i, jsl, r] = 1 iff i == (r//64)*4 + jsl  (same for both head pairs)
    ind = np.zeros((8, 4, 128), dtype=np.float32)
    for jsl in range(4):
        for r in range(128):
            ind[(r // 64) * 4 + jsl, jsl, r] = 1.0
    ind = ind.astype(BF16)

    perm = np.concatenate([np.arange(0, DK, 2), np.arange(1, DK, 2)])  # evens,odds

    in_maps = []
    for core in range(NCORES):
        b, hg = divmod(core, 4)
        ch0 = hg * 256
        qk_rows = np.concatenate([ch0 + hl * 64 + perm for hl in range(HPC)])
        in_maps.append({
            "xt": np.ascontiguousarray(np.asarray(x[b]).T).astype(BF16),
            "wq": np.ascontiguousarray(np.asarray(WQ)[qk_rows, :].T).astype(BF16),
            "wk": np.ascontiguousarray(np.asarray(WK)[qk_rows, :].T).astype(BF16),
            "wv": np.ascontiguousarray(np.asarray(WV)[ch0:ch0 + 256, :].T).astype(BF16),
            "wo": np.ascontiguousarray(np.asarray(WO)[:, ch0:ch0 + 256].T).astype(BF16),
            "cosb": cosb,
            "sinb": sinb,
            "msk": msk,
            "eye": eye,
            "ind": ind,
        })
    return in_maps


LAST_EXEC_NS = None


def kernel(x, token_positions, WQ, WK, WV, WO):
    global LAST_EXEC_NS
    from concourse.bass_utils import run_bass_kernel_spmd

    if "nc" not in _COMPILED:
        _COMPILED["nc"] = _build_nc()
    nc = _COMPILED["nc"]

    in_maps = _host_prep(x, token_positions, WQ, WK, WV, WO)
    res = run_bass_kernel_spmd(nc, in_maps, list(range(NCORES)))
    LAST_EXEC_NS = res.exec_time_ns

    out = np.zeros((2, S, D), dtype=np.float32)
    for core in range(NCORES):
        out[core // 4] += np.asarray(res.results[core]["out"], dtype=np.float32)
    return out
